# revision 11
# baseline (speedup 1.0000x reference)
"""Trainium2 Bass kernel for an Elman RNN (nn_RNN_28561532518804).

  xh = x @ W_xh.T + b_xh                       # [S, B, H] input GEMM
  h_t = tanh(xh_t + h_{t-1} @ W_hh.T + b_hh)   # sequential scan, 512 steps
  y  = hs @ W_hy.T + b_hy                      # [S, B, O] output GEMM

Sharding: pure data-parallel over batch (B=64 -> 8 per core), no
collectives.  Everything on-chip is feature-major ("transposed") so the
sequential recurrence needs no per-step transposes.

Per step the recurrence runs 64 (ldweights, matmul N=8) pairs — the PE is
bound by streaming W_hh.T through the array via fast-weight-load
(~27-31 ns per 128x128 bf16 tile).  The contraction is split 3/5 into two
PSUM tiles per output half so tanh of h-blocks 0..3 completes while the PE
is still on the second half, and the next step's first matmuls (which only
need blocks 0..2) start under tanh of blocks 4..7.

The two big GEMMs are *interleaved into the recurrence*: their matmuls
(N=512, one PSUM bank) are inserted two-per-step after each step's last
recurrence matmul, where the PE would otherwise idle waiting on the
add->tanh chain.  The h trajectory stays in SBUF (two 64-step window
buffers, ping-pong); window w's output GEMM runs during window w+1.

Compute dtype bf16 with fp32 PSUM accumulation: measured end-to-end
relative error vs the fp32 reference is ~3.5e-3.
"""

import numpy as np
import ml_dtypes

import concourse.bass as bass
import concourse.mybir as mybir
import concourse.tile as tile
from concourse import bacc
from concourse.bass_utils import run_bass_kernel_spmd

S, B, I, H, O = 512, 64, 1024, 1024, 1024
P = 128
NCORES = 8
BC = B // NCORES            # 8 batch rows per core
KT = H // P                 # 8 tiles along any 1024 feature dim
WT = 64                     # timesteps per window (= GEMM chunk of 512 cols)
NCH = WT * BC               # 512 moving columns per GEMM chunk
KSPLIT = 4                  # recurrence contraction split: A=k[0..4) B=k[4..8)

F32 = mybir.dt.float32
BF16 = mybir.dt.bfloat16
BF = ml_dtypes.bfloat16
AF = mybir.ActivationFunctionType


def _build(s_steps: int = S):
    """Build + compile the single-core program (identical on all cores)."""
    from contextlib import ExitStack

    sb = s_steps * BC
    assert s_steps % WT == 0 or s_steps < WT
    n_win = max(1, s_steps // WT)
    wt = min(WT, s_steps)

    nch = wt * BC
    nc = bacc.Bacc(
        "TRN2", target_bir_lowering=False, debug=False, num_devices=NCORES
    )
    xT = nc.dram_tensor("xT", [I, sb], BF16, kind="ExternalInput").ap()
    wxT = nc.dram_tensor("wxT", [I, H], BF16, kind="ExternalInput").ap()
    whT = nc.dram_tensor("whT", [H, H], BF16, kind="ExternalInput").ap()
    wyT = nc.dram_tensor("wyT", [H, O], BF16, kind="ExternalInput").ap()
    bxh = nc.dram_tensor("bxh", [H, 1], F32, kind="ExternalInput").ap()  # b_xh+b_hh
    bhy = nc.dram_tensor("bhy", [O, 1], F32, kind="ExternalInput").ap()
    yT = nc.dram_tensor("yT", [O, sb], F32, kind="ExternalOutput").ap()

    with tile.TileContext(nc) as tc, ExitStack() as ctx:
        wpool = ctx.enter_context(tc.tile_pool(name="weights", bufs=1))
        xhpool = ctx.enter_context(tc.tile_pool(name="xh", bufs=1))
        win_pool = ctx.enter_context(tc.tile_pool(name="win", bufs=1))
        xin_pool = ctx.enter_context(tc.tile_pool(name="xin", bufs=2))
        y_pool = ctx.enter_context(tc.tile_pool(name="yout", bufs=3))
        ps_big = ctx.enter_context(tc.tile_pool(name="psbig", bufs=1, space="PSUM"))
        ps_sm = ctx.enter_context(tc.tile_pool(name="pssm", bufs=6, space="PSUM"))
        u_pool = ctx.enter_context(tc.tile_pool(name="upool", bufs=4))

        # ---- weights + biases ------------------------------------------
        wx_sb, wh_sb, wy_sb = [], [], []
        for k in range(KT):
            wx = wpool.tile([P, H], BF16, name=f"wx{k}")
            nc.sync.dma_start(wx[:], wxT[k * P : (k + 1) * P, :])
            wx_sb.append(wx)
        bxh_sb = wpool.tile([P, KT], F32, name="bxh_sb")
        nc.sync.dma_start(bxh_sb[:], bxh.rearrange("(m p) o -> p (m o)", p=P))
        bhy_sb = wpool.tile([P, KT], F32, name="bhy_sb")
        nc.sync.dma_start(bhy_sb[:], bhy.rearrange("(m p) o -> p (m o)", p=P))

        # xh packed like the h state: column t*64 + mi*8 + b
        xh_all = xhpool.tile([P, sb * KT], BF16, name="xh_all")
        xh_r = xh_all.rearrange("p (t m b) -> p t (m b)", m=KT, b=BC)
        xT_r = xT.rearrange("(k p) n -> p k n", p=P)

        # h trajectory windows (SBUF-resident, ping-pong)
        win = [win_pool.tile([P, wt * B], BF16, name=f"win{i}") for i in range(2)]
        win_r = [w.rearrange("p (t k b) -> p t (k b)", k=KT, b=BC) for w in win]
        h0z = win_pool.tile([P, B], BF16, name="h0z")
        nc.gpsimd.memset(h0z[:], 0.0)

        xt_tiles = {}

        def load_x_chunk(c):
            xt = xin_pool.tile([P, KT, nch], BF16, tag="xin", name="xin")
            nc.sync.dma_start(xt[:], xT_r[:, :, c * nch : (c + 1) * nch])
            xt_tiles[c] = xt

        # phase-1 GEMM pieces for chunk c: 64 matmuls (mi-major), evictions
        # after each 8-MM group.  `mm_range` selects which to emit now.
        p1_ps = {}

        def phase1_mms(c, lo, hi):
            xt = xt_tiles[c]
            for j in range(lo, hi):
                mi, k = divmod(j, KT)
                if k == 0:
                    p1_ps[c] = ps_big.tile([P, nch], F32, tag="ps1", name="ps1")
                nc.tensor.matmul(
                    p1_ps[c][:],
                    wx_sb[k][:, mi * P : (mi + 1) * P],
                    xt[:, k, :],
                    start=(k == 0),
                    stop=(k == KT - 1),
                )
                if k == KT - 1:
                    nc.scalar.activation(
                        xh_r[:, c * wt : (c + 1) * wt, mi * BC : (mi + 1) * BC],
                        p1_ps[c][:],
                        AF.Identity,
                        bias=bxh_sb[:, mi : mi + 1],
                    )

        # phase-3 GEMM pieces for window w (reads win[w % 2])
        p3_ps = {}

        def phase3_mms(w, lo, hi):
            wr = win_r[w % 2]
            for j in range(lo, hi):
                mo, k = divmod(j, KT)
                if k == 0:
                    p3_ps[w] = ps_big.tile([P, nch], F32, tag="ps3", name="ps3")
                nc.tensor.matmul(
                    p3_ps[w][:],
                    wy_sb[k][:, mo * P : (mo + 1) * P],
                    wr[:, :, k * BC : (k + 1) * BC],
                    start=(k == 0),
                    stop=(k == KT - 1),
                )
                if k == KT - 1:
                    yt = y_pool.tile([P, nch], F32, tag="y", name="yt")
                    nc.scalar.activation(
                        yt[:], p3_ps[w][:], AF.Identity, bias=bhy_sb[:, mo : mo + 1]
                    )
                    nc.sync.dma_start(
                        yT[mo * P : (mo + 1) * P, w * nch : (w + 1) * nch], yt[:]
                    )

        # ---- startup: x chunk 0 + dedicated phase-1 chunk 0 -------------
        load_x_chunk(0)
        phase1_mms(0, 0, 8 * KT)
        for k in range(KT):
            wh = wpool.tile([P, H], BF16, name=f"wh{k}")
            nc.sync.dma_start(wh[:], whT[k * P : (k + 1) * P, :])
            wh_sb.append(wh)
        for k in range(KT):
            wy = wpool.tile([P, H], BF16, name=f"wy{k}")
            nc.sync.dma_start(wy[:], wyT[k * P : (k + 1) * P, :])
            wy_sb.append(wy)

        # ---- recurrence with interleaved GEMM work ----------------------
        HB = B // 2
        kA = list(range(KSPLIT))
        kB = list(range(KSPLIT, KT))

        def _mm_quarter(ps, cur, g, ks):
            for mi in range(4 * g, 4 * g + 4):
                for j, k in enumerate(ks):
                    nc.tensor.matmul(
                        ps[:, (mi % 4) * BC : (mi % 4 + 1) * BC],
                        wh_sb[k][:, mi * P : (mi + 1) * P],
                        cur[:, k * BC : (k + 1) * BC],
                        start=(j == 0),
                        stop=(j == len(ks) - 1),
                    )

        for t in range(s_steps):
            c, j = divmod(t, wt)
            if t == 0:
                cur = h0z[:]
            else:
                pc, pj = divmod(t - 1, wt)
                cur = win[pc % 2][:, pj * B : (pj + 1) * B]
            nxt = win[c % 2]
            jo = j * B

            psA0 = ps_sm.tile([P, HB], F32, tag="ps2", name="psA0")
            psA1 = ps_sm.tile([P, HB], F32, tag="ps2", name="psA1")
            psB0 = ps_sm.tile([P, HB], F32, tag="ps2", name="psB0")
            psB1 = ps_sm.tile([P, HB], F32, tag="ps2", name="psB1")
            _mm_quarter(psA0, cur, 0, kA)
            _mm_quarter(psA1, cur, 1, kA)
            vA0 = u_pool.tile([P, HB], F32, tag="v", name="vA0")
            nc.vector.tensor_add(vA0[:], psA0[:], xh_all[:, t * B : t * B + HB])
            vA1 = u_pool.tile([P, HB], F32, tag="v", name="vA1")
            nc.vector.tensor_add(vA1[:], psA1[:], xh_all[:, t * B + HB : (t + 1) * B])
            # interleaved GEMM work in the tanh-chain shadow: one inserted
            # matmul between B0 and B1 (stretches PE-busy past the tanh0
            # chain), one after B1.  First half-window: output GEMM of the
            # previous window; second half: input GEMM of the next chunk.
            if j == 0 and c + 1 < n_win:
                load_x_chunk(c + 1)

            def _insert(slot):  # slot 0 or 1
                if j < wt // 2:
                    if c >= 1:
                        phase3_mms(c - 1, 2 * j + slot, 2 * j + slot + 1)
                else:
                    if c + 1 < n_win:
                        jj = j - wt // 2
                        phase1_mms(c + 1, 2 * jj + slot, 2 * jj + slot + 1)

            _mm_quarter(psB0, cur, 0, kB)
            u0 = u_pool.tile([P, HB], F32, tag="u", name="u0")
            nc.vector.tensor_add(u0[:], vA0[:], psB0[:])
            nc.scalar.activation(nxt[:, jo : jo + HB], u0[:], AF.Tanh)
            _insert(0)
            _mm_quarter(psB1, cur, 1, kB)
            u1 = u_pool.tile([P, HB], F32, tag="u", name="u1")
            nc.vector.tensor_add(u1[:], vA1[:], psB1[:])
            nc.scalar.activation(nxt[:, jo + HB : jo + B], u1[:], AF.Tanh)
            _insert(1)

        # ---- tail: output GEMM of the last window -----------------------
        phase3_mms(n_win - 1, 0, 8 * KT)

    nc.compile()
    return nc


_cache: dict = {}


def _get_nc(s_steps: int):
    if s_steps not in _cache:
        _cache[s_steps] = _build(s_steps)
    return _cache[s_steps]


def _prep_inputs(x, W_xh, b_xh, W_hh, b_hh, W_hy, b_hy):
    s_steps = x.shape[0]
    wxT = np.ascontiguousarray(np.asarray(W_xh, np.float32).T).astype(BF)
    whT = np.ascontiguousarray(np.asarray(W_hh, np.float32).T).astype(BF)
    wyT = np.ascontiguousarray(np.asarray(W_hy, np.float32).T).astype(BF)
    bxh = np.asarray(np.asarray(b_xh, np.float32) + np.asarray(b_hh, np.float32)).reshape(H, 1)
    bhy = np.asarray(b_hy, np.float32).reshape(O, 1)
    in_maps = []
    for c in range(NCORES):
        xc = np.asarray(x[:, c * BC : (c + 1) * BC, :], np.float32)
        xTc = np.ascontiguousarray(xc.transpose(2, 0, 1).reshape(I, s_steps * BC))
        in_maps.append(
            {
                "xT": xTc.astype(BF),
                "wxT": wxT,
                "whT": whT,
                "wyT": wyT,
                "bxh": bxh,
                "bhy": bhy,
            }
        )
    return in_maps


def run(x, W_xh, b_xh, W_hh, b_hh, W_hy, b_hy, trace=False, **spmd_kwargs):
    s_steps = x.shape[0]
    nc = _get_nc(s_steps)
    in_maps = _prep_inputs(x, W_xh, b_xh, W_hh, b_hh, W_hy, b_hy)
    res = run_bass_kernel_spmd(
        nc, in_maps, core_ids=list(range(NCORES)), trace=trace, **spmd_kwargs
    )
    y = np.empty((s_steps, B, O), np.float32)
    for c in range(NCORES):
        yTc = np.asarray(res.results[c]["yT"], np.float32)
        y[:, c * BC : (c + 1) * BC, :] = yTc.reshape(O, s_steps, BC).transpose(1, 2, 0)
    return y, res


def kernel(x, W_xh, b_xh, W_hh, b_hh, W_hy, b_hy):
    y, _ = run(x, W_xh, b_xh, W_hh, b_hh, W_hy, b_hy)
    return y
